# revision 40
# baseline (speedup 1.0000x reference)
"""Differential attention (GQA + RoPE) Bass/Tile kernel for 8 TRN2 NeuronCores.

Sharding: tensor-parallel over the 16 query heads (2 per core, kv head c//2),
Wq/Wk/Wv column-sharded per core; attention output exchanged with an on-device
AllToAll into sequence shards; o_proj row-parallel per sequence shard with the
full Wo on every core; host concatenates the 8 row shards.

Layout notes:
 - x is passed transposed (xT [D, S]) so the contraction dim of every
   projection matmul lands on SBUF partitions.
 - Wq/Wk columns are permuted per head so RoPE's interleaved complex pairs
   become contiguous blocks [x0_A | x1_A | x0_B | x1_B] (A = freqs 0..31,
   B = freqs 32..63).  Attention scores are invariant to any per-half channel
   permutation applied consistently to q and k.
 - Softmax is computed max-free (scores for this problem are within ±6, far
   inside fp16/exp range); the row sum rides the AV matmul as an extra N=1
   matmul against a ones vector, reusing the loaded P^T stationary tile.
 - a1 - lam*a2 is folded linearly: out = (u1*inv_r1 - lam*u2*inv_r2) * 0.5.
"""

import numpy as np
from contextlib import ExitStack

import concourse.bacc as bacc
import concourse.tile as tile
from concourse import mybir
from concourse.bass_utils import run_bass_kernel_spmd

S = 2048
D = 2048
H = 16
KV = 4
HD = 128
HALF = 64
NCORES = 8
HPC = H // NCORES      # 2 query heads per core
P = 128
NT = S // P            # 16 tiles of 128 along s/t
NSC = 4                # s-chunks of 512
SCW = 512
DT = D // P            # 16 tiles along contraction dim
SROWS = S // NCORES    # 256 output rows per core
SCALE = 1.0 / 8.0      # 1/sqrt(HALF)
OUT_SCALE = 0.5        # 1 - lambda_init
NEG = -1.0e9

f32 = mybir.dt.float32
f32r = mybir.dt.float32r
f16 = mybir.dt.float16

_CACHE = {}


def _build():
    nc = bacc.Bacc("TRN2", target_bir_lowering=False, debug=False,
                   num_devices=NCORES)
    xT = nc.declare_dram_parameter("xT", [D, S], f16, isOutput=False)
    wall = nc.declare_dram_parameter("wall", [D, 4 * P], f16, isOutput=False)
    wo = nc.declare_dram_parameter("wo", [D, D], f16, isOutput=False)
    cosT = nc.declare_dram_parameter("cosT", [P, S], f16, isOutput=False)
    sinT = nc.declare_dram_parameter("sinT", [P, S], f16, isOutput=False)
    masks = nc.declare_dram_parameter("masks", [P, 4 * SCW], f16, isOutput=False)
    ident = nc.declare_dram_parameter("ident", [P, P], f32, isOutput=False)
    lam = nc.declare_dram_parameter("lam", [1, HPC], f32, isOutput=False)
    o_out = nc.declare_dram_parameter("o_out", [SROWS, D], f32, isOutput=True)

    rg = [list(range(NCORES))]

    with tile.TileContext(nc) as tc, ExitStack() as ctx:
        const = ctx.enter_context(tc.tile_pool(name="const", bufs=1))
        dram = ctx.enter_context(tc.tile_pool(name="dram", bufs=1, space="DRAM"))

        # rows 0..63 and 64..127 both hold freqs 0..63, so every rope
        # operand pair sees equal SBUF base partitions
        cos_sb = const.tile([P, S], f16)
        nc.gpsimd.dma_start(out=cos_sb[:, :], in_=cosT[:, :])
        sin_sb = const.tile([P, S], f16)
        nc.gpsimd.dma_start(out=sin_sb[:, :], in_=sinT[:, :])
        mask_sb = const.tile([P, 4 * SCW], f16)
        nc.gpsimd.dma_start(out=mask_sb[:, :], in_=masks[:, :])
        id_sb = const.tile([P, P], f32)
        nc.gpsimd.dma_start(out=id_sb[:, :], in_=ident[:, :])
        lam_sb = const.tile([1, HPC], f32)
        nc.gpsimd.dma_start(out=lam_sb[:, :], in_=lam[:, :])
        lam_sig = const.tile([1, HPC], f32)
        nc.scalar.activation(lam_sig[:, :], lam_sb[:, :],
                             mybir.ActivationFunctionType.Sigmoid)
        lam_b = const.tile([P, HPC], f32)
        nc.gpsimd.partition_broadcast(lam_b[:, :], lam_sig[:, :])

        # Persistent per-core tensors: projected qT/kT (rope applied) per head,
        # kT, vT (fp32, channel-major) and v16 (fp16, t-major for AV rhs).
        qkvp = ctx.enter_context(tc.tile_pool(name="qkvp", bufs=1))
        qkv = [qkvp.tile([P, S], f16, name=f"qkv{j}") for j in range(2)]
        # k with only half-A (resp. half-B) channels non-zero, so the score
        # matmuls contract over the full 128 partitions with no slicing
        kA = qkvp.tile([P, S], f16, name="kA")
        kB = qkvp.tile([P, S], f16, name="kB")
        nc.vector.memset(kA[:, :], 0.0)
        nc.vector.memset(kB[:, :], 0.0)
        vT32 = qkvp.tile([P, S], f32)
        # v in t-major fp16, one 136-wide group per t-tile:
        # cols [136jt, 136jt+128) = v, col 136jt+128 = 2.0 -- the rowsum
        # rider; pre-doubled sums make their reciprocal carry the 0.5 scale
        VG = 136
        v16 = qkvp.tile([P, NT * VG], f16)
        nc.vector.memset(v16[:, :].rearrange("p (jt g) -> p jt g", g=VG)
                         [:, :, 128:129], 2.0)

        # ---- Stage 1: fused qkv projection (+RoPE on eviction) ----
        with tc.tile_pool(name="wall_p", bufs=1) as wall_pool, \
             tc.tile_pool(name="xt_p", bufs=2) as xt_pool, \
             tc.tile_pool(name="rtmp", bufs=4) as rtmp, \
             tc.tile_pool(name="qscr", bufs=6) as qscr, \
             tc.tile_pool(name="ps1", bufs=4, space="PSUM") as ps1:
            # one DMA for the whole weight block: column group dt holds
            # wall[dt*128:(dt+1)*128, :] -> [128, 16*512]
            w_sb = wall_pool.tile([P, DT * 4 * P], f16, name="w_sb")
            for g in range(4):
                gdt = slice(g * 4, (g + 1) * 4)
                nc.scalar.dma_start(
                    out=w_sb[:, g * 4 * 4 * P:(g + 1) * 4 * 4 * P].rearrange(
                        "p (dt j) -> p dt j", dt=4),
                    in_=wall.ap().rearrange("(dt p) j -> p dt j", p=P)
                    [:, gdt, :])

            for sc in range(NSC):
                # one DMA per s-chunk: column group dt holds
                # xT[dt*128:(dt+1)*128, sc*512:(sc+1)*512]
                xts = xt_pool.tile([P, DT * SCW], f16, name="xt", tag="xt")
                for g in range(4):
                    gdt = slice(g * 4, (g + 1) * 4)
                    nc.sync.dma_start(
                        out=xts[:, g * 4 * SCW:(g + 1) * 4 * SCW].rearrange(
                            "p (dt f) -> p dt f", dt=4),
                        in_=xT[:, sc * SCW:(sc + 1) * SCW].rearrange(
                            "(dt p) f -> p dt f", p=P)[:, gdt, :])
                for j in (2, 0, 1, 3):
                    psum_p = ps1.tile([P, SCW], f32, name="psum_p", tag="p1")
                    for dt_ in range(DT):
                        nc.tensor.matmul(
                            psum_p[:, :],
                            w_sb[:, dt_ * 4 * P + j * P:
                                 dt_ * 4 * P + (j + 1) * P],
                            xts[:, dt_ * SCW:(dt_ + 1) * SCW],
                            start=(dt_ == 0), stop=(dt_ == DT - 1))
                    ssl = slice(sc * SCW, (sc + 1) * SCW)
                    # psum rows: [x0 (freqs 0..63) | x1 (freqs 0..63)]
                    tags = ("t0", "t1") if j < 2 else ("kt0", "kt1")
                    t0 = rtmp.tile([P, SCW], f16, name="t0", tag=tags[0])
                    t1 = rtmp.tile([P, SCW], f16, name="t1", tag=tags[1])
                    if j < 3:
                        # evict on the scalar engine so the PSUM bank frees
                        # fast; rope then runs SBUF->SBUF on DVE
                        xsc = qscr.tile([P, SCW], f16, name="xsc", tag="xsc")
                        nc.scalar.copy(xsc[:, :], psum_p[:, :])
                        psum_p = xsc
                    if j < 2:
                        # q rope at full 64-row width:
                        # rows 0..63 = r0, rows 64..127 = r1
                        q_t = qkv[j]
                        nc.vector.tensor_mul(t0[0:64, :], psum_p[64:128, :],
                                             sin_sb[64:128, ssl])
                        nc.vector.tensor_mul(q_t[0:64, ssl], psum_p[0:64, :],
                                             cos_sb[0:64, ssl])
                        nc.vector.tensor_sub(q_t[0:64, ssl], q_t[0:64, ssl],
                                             t0[0:64, :])
                        nc.vector.tensor_mul(t1[64:128, :], psum_p[0:64, :],
                                             sin_sb[0:64, ssl])
                        nc.vector.tensor_mul(q_t[64:128, ssl],
                                             psum_p[64:128, :],
                                             cos_sb[64:128, ssl])
                        nc.vector.tensor_add(q_t[64:128, ssl],
                                             q_t[64:128, ssl], t1[64:128, :])
                    elif j == 2:
                        # k rope scattered into kA (half-A rows) / kB (half-B)
                        for hf, kt in ((0, kA), (1, kB)):
                            fr = slice(32 * hf, 32 * hf + 32)
                            r1 = slice(64 + 32 * hf, 64 + 32 * hf + 32)
                            x0 = psum_p[fr, :]
                            x1 = psum_p[r1, :]
                            eng = nc.vector
                            eng.tensor_mul(t0[fr, :], x1, sin_sb[r1, ssl])
                            eng.tensor_mul(kt[fr, ssl], x0, cos_sb[fr, ssl])
                            eng.tensor_sub(kt[fr, ssl], kt[fr, ssl],
                                           t0[fr, :])
                            eng.tensor_mul(t1[r1, :], x0, sin_sb[fr, ssl])
                            eng.tensor_mul(kt[r1, ssl], x1, cos_sb[r1, ssl])
                            eng.tensor_add(kt[r1, ssl], kt[r1, ssl],
                                           t1[r1, :])
                    else:
                        nc.scalar.copy(vT32[:, ssl], psum_p[:, :])

        # ---- Stage 1.5: transpose v to t-major fp16 ----
        with tc.tile_pool(name="pst", bufs=1, space="PSUM") as pst:
            for jt in range(NT):
                ps_t = pst.tile([P, P], f32, name="ps_vt", tag="vt")
                nc.tensor.transpose(ps_t[:, :],
                                    vT32[:, jt * P:(jt + 1) * P], id_sb[:, :])
                nc.scalar.copy(v16[:, jt * 136:jt * 136 + P], ps_t[:, :])

        # prefetch full Wo (fp16) early on the scalar DMA queue so the
        # o_proj partials can start the moment the first AllToAll lands
        wo_pool = ctx.enter_context(tc.tile_pool(name="wo_p", bufs=1))
        wos_l = []
        for dc in range(4):
            wos = wo_pool.tile([P, H * SCW], f16, name="wos", tag=f"wos{dc}")
            nc.gpsimd.dma_start(
                out=wos[:, :].rearrange("p (ht f) -> p ht f", ht=H),
                in_=wo[:, dc * SCW:(dc + 1) * SCW].rearrange(
                    "(ht p) f -> p ht f", p=P))
            wos_l.append(wos)

        # ---- Stage 2: differential attention per (head, s-chunk) ----
        attnp = ctx.enter_context(tc.tile_pool(name="attnp", bufs=1))
        attnT = [attnp.tile([P, S], f16, name=f"attnT{h}") for h in range(HPC)]

        # per-head AllToAll bounce buffers (issued as soon as head h is done,
        # so the first exchange overlaps the second head's compute)
        sec = P * SROWS  # elems per (core, head) section
        bounce_in = [dram.tile([NCORES * sec], f16, name=f"bounce_in{h}")
                     for h in range(HPC)]
        bounce_out = [dram.tile([NCORES * sec], f16, name=f"bounce_out{h}")
                      for h in range(HPC)]

        misc_ps = ctx.enter_context(
            tc.tile_pool(name="misc_ps", bufs=2, space="PSUM"))
        with tc.tile_pool(name="expst_p", bufs=2) as expst_pool, \
             tc.tile_pool(name="st_p", bufs=2, space="PSUM") as st_pool, \
             tc.tile_pool(name="u_p", bufs=1, space="PSUM") as u_pool, \
             tc.tile_pool(name="cmb", bufs=4) as cmb:
            UG = 136
            for h in range(HPC):
                for sc in range(NSC):
                    njt = 4 * sc + 4
                    ps_u = [None, None]
                    for hf in range(2):
                        k_t = kA if hf == 0 else kB
                        expst = expst_pool.tile([P, njt * SCW], f16,
                                                name="expst", tag="expst",
                                                bufs=3)
                        for jt in range(njt):
                            ps_st = st_pool.tile([P, SCW], f32,
                                                 name="ps_st", tag="st")
                            nc.tensor.matmul(
                                ps_st[:, :],
                                k_t[:, jt * P:(jt + 1) * P],
                                qkv[h][:, sc * SCW:(sc + 1) * SCW],
                                start=True, stop=True)
                            # columns below 128*m are never read by the AV
                            # loop (fully-masked): restrict mask+exp to live.
                            # Causality is applied as a 0/1 fp16 multiply on
                            # the exp output (cheaper on DVE than the fp32
                            # PSUM -1e9 add; the rowsum rider sums the same
                            # zeroed tile, so the math is unchanged).
                            lo = 0
                            if jt >= 4 * sc:
                                m = jt % 4
                                lo = P * m
                            esl = expst[:, jt * SCW + lo:(jt + 1) * SCW]
                            nc.scalar.activation(
                                esl, ps_st[:, lo:],
                                mybir.ActivationFunctionType.Exp, scale=SCALE)
                            if jt >= 4 * sc and lo < SCW:
                                nc.vector.tensor_mul(
                                    esl, esl,
                                    mask_sb[:, m * SCW + lo:(m + 1) * SCW])
                        # two banks per half: [u(128) | r(1) | pad] x2
                        ps_u[hf] = [
                            u_pool.tile([P, 2 * UG], f32,
                                        name=f"ps_u{hf}{qq}", tag=f"u{hf}{qq}")
                            for qq in range(2)]
                        for q_ in range(4):
                            js = 4 * sc + q_
                            put = ps_u[hf][q_ // 2]
                            off = UG * (q_ % 2)
                            for jt in range(js + 1):
                                lhs = expst[:, jt * SCW + q_ * P:
                                            jt * SCW + q_ * P + P]
                                nc.tensor.matmul(
                                    put[:, off:off + 129],
                                    lhs, v16[:, jt * VG:jt * VG + 129],
                                    start=(jt == 0), stop=(jt == js))
                    # combine: attn = 0.5*(u1*inv_r1 - lam*u2*inv_r2)
                    inv = cmb.tile([P, 8], f32, name="inv", tag="inv")
                    for hf in range(2):
                        for q_ in range(4):
                            nc.vector.reciprocal(
                                inv[:, 4 * hf + q_:4 * hf + q_ + 1],
                                ps_u[hf][q_ // 2][:, UG * (q_ % 2) + 128:
                                                  UG * (q_ % 2) + 129])
                    for q_ in range(4):
                        js = 4 * sc + q_
                        u0 = ps_u[0][q_ // 2][:, UG * (q_ % 2):
                                              UG * (q_ % 2) + P]
                        u1 = ps_u[1][q_ // 2][:, UG * (q_ % 2):
                                              UG * (q_ % 2) + P]
                        sc2 = cmb.tile([P, 1], f32, name="sc2", tag="sc2")
                        nc.vector.tensor_scalar_mul(
                            sc2[:, :], inv[:, 4 + q_:5 + q_],
                            lam_b[:, h:h + 1])
                        tmp2 = cmb.tile([P, P], f32, name="tmp2", tag="tmp2")
                        nc.vector.tensor_scalar_mul(tmp2[:, :], u1, sc2[:, :])
                        attn_sl = cmb.tile([P, P], f32, name="attn_sl",
                                           tag="attn_sl")
                        nc.vector.scalar_tensor_tensor(
                            attn_sl[:, :], u0,
                            inv[:, q_:q_ + 1], tmp2[:, :],
                            mybir.AluOpType.mult, mybir.AluOpType.subtract)
                        ps_t = misc_ps.tile([P, P], f32, name="ps_at",
                                            tag="misc")
                        nc.tensor.transpose(ps_t[:, :], attn_sl[:, :],
                                            id_sb[:, :])
                        nc.vector.tensor_copy(attnT[h][:, js * P:(js + 1) * P],
                                              ps_t[:, :])
                # ---- Stage 3 (per head): AllToAll into sequence shards ----
                if sc == NSC - 1:
                    nc.gpsimd.dma_start(
                        out=bounce_in[h][:].rearrange(
                            "(d p f) -> p d f", d=NCORES, f=SROWS),
                        in_=attnT[h][:, :].rearrange(
                            "p (d f) -> p d f", f=SROWS))
                    nc.gpsimd.collective_compute(
                        "AllToAll", mybir.AluOpType.bypass, replica_groups=rg,
                        ins=[bounce_in[h][:]], outs=[bounce_out[h][:]])

        # ---- Stage 4: o_proj over the local 256 rows ----
        with tc.tile_pool(name="aT_p", bufs=1) as aT_pool, \
             tc.tile_pool(name="o_p", bufs=4) as o_pool, \
             tc.tile_pool(name="ps4", bufs=2, space="PSUM") as ps4:
            aTl = []
            for h in range(HPC):
                a_t = aT_pool.tile([P, NCORES * SROWS], f16, name=f"aT{h}")
                nc.gpsimd.dma_start(
                    out=a_t[:, :].rearrange("p (d f) -> p d f", d=NCORES),
                    in_=bounce_out[h][:].rearrange(
                        "(d p f) -> p d f", d=NCORES, f=SROWS))
                aTl.append(a_t)
            # head-0 sections only need the first AllToAll: run ALL of their
            # partial o_proj groups while the second exchange is in flight.
            o_es = {}
            for dc in range(4):
                for st_ in range(2):
                    ps_e = misc_ps.tile([P, SCW], f32, name="ps_e", tag="misc")
                    for i, ht in enumerate(range(0, H, 2)):
                        nc.tensor.matmul(
                            ps_e[:, :],
                            aTl[0][:, (ht // 2) * SROWS + st_ * P:
                                   (ht // 2) * SROWS + (st_ + 1) * P],
                            wos_l[dc][:, ht * SCW:(ht + 1) * SCW],
                            start=(i == 0), stop=(i == H // 2 - 1))
                    o_e = o_pool.tile([P, SCW], f32, name="o_e",
                                      tag=f"o_e{dc}{st_}", bufs=1)
                    nc.vector.tensor_copy(o_e[:, :], ps_e[:, :])
                    o_es[(dc, st_)] = o_e
            # keep the PE (HAM) warm across the second AllToAll's wait
            # window so the head-1 pass below starts at full clock; the
            # result is parked in DRAM and never read.
            warm_ps = misc_ps.tile([P, SCW], f32, name="warm", tag="misc")
            for w_ in range(40):
                nc.tensor.matmul(warm_ps[:, :], v16[:, 0:P],
                                 aTl[0][:, 0:SCW], start=True, stop=True)
            warm_sb = o_pool.tile([P, SCW], f32, name="warm_sb",
                                  tag="warm_sb", bufs=1)
            nc.vector.tensor_copy(warm_sb[:, :], warm_ps[:, :])
            warm_dram = dram.tile([P * SCW], f32, name="warm_dram")
            nc.gpsimd.dma_start(
                out=warm_dram[:].rearrange("(p f) -> p f", f=SCW),
                in_=warm_sb[:, :])
            for dc in range(4):
                for st_ in range(2):
                    ps_o = ps4.tile([P, SCW], f32, name="ps_o", tag="o")
                    for i, ht in enumerate(range(1, H, 2)):
                        nc.tensor.matmul(
                            ps_o[:, :],
                            aTl[1][:, (ht // 2) * SROWS + st_ * P:
                                   (ht // 2) * SROWS + (st_ + 1) * P],
                            wos_l[dc][:, ht * SCW:(ht + 1) * SCW],
                            start=(i == 0), stop=(i == H // 2 - 1))
                    o_sb = o_pool.tile([P, SCW], f32, name="o_sb", tag="o_sb")
                    nc.vector.tensor_add(o_sb[:, :], ps_o[:, :],
                                         o_es[(dc, st_)][:, :])
                    nc.sync.dma_start(
                        out=o_out[st_ * P:(st_ + 1) * P,
                                  dc * SCW:(dc + 1) * SCW],
                        in_=o_sb[:, :])

    nc.compile()
    return nc


def _prep(x, freqs_cos, freqs_sin, Wq, Wk, Wv, Wo, lambda_param):
    """Host-side sharding/layout prep. Returns per-core input maps."""
    x2 = np.asarray(x, np.float32).reshape(S, D)
    xT = np.ascontiguousarray(x2.T.astype(np.float16))
    cosT = np.asarray(freqs_cos, np.float32).T
    sinT = np.asarray(freqs_sin, np.float32).T
    cosT = np.ascontiguousarray(
        np.concatenate([cosT, cosT], axis=0).astype(np.float16))
    sinT = np.ascontiguousarray(
        np.concatenate([sinT, sinT], axis=0).astype(np.float16))
    Wq = np.asarray(Wq, np.float32)
    Wk = np.asarray(Wk, np.float32)
    Wv = np.asarray(Wv, np.float32)
    Wo16 = np.ascontiguousarray(np.asarray(Wo, np.float32).astype(np.float16))
    lamp = np.asarray(lambda_param, np.float32)

    # de-interleave complex pairs: [x0 (freqs 0..63) | x1 (freqs 0..63)]
    perm = np.concatenate([
        2 * np.arange(64), 2 * np.arange(64) + 1]).astype(np.int64)

    # causal mask variants for the 4 in-chunk diagonal positions
    t_rel = np.arange(P)[:, None]
    s_rel = np.arange(SCW)[None, :]
    mask_all = np.empty((P, 4 * SCW), np.float16)
    for m in range(4):
        mask_all[:, m * SCW:(m + 1) * SCW] = np.where(
            P * m + t_rel <= s_rel, 1.0, 0.0)

    ident = np.eye(P, dtype=np.float32)

    in_maps = []
    for c in range(NCORES):
        g = c // 2
        cols = []
        for h in (2 * c, 2 * c + 1):
            cols.append(Wq[:, h * HD:(h + 1) * HD][:, perm])
        cols.append(Wk[:, g * HD:(g + 1) * HD][:, perm])
        cols.append(Wv[:, g * HD:(g + 1) * HD])
        wall = np.ascontiguousarray(
            np.concatenate(cols, axis=1).astype(np.float16))
        in_maps.append({
            "xT": xT,
            "wall": wall,
            "wo": Wo16,
            "cosT": cosT,
            "sinT": sinT,
            "masks": mask_all,
            "ident": ident,
            "lam": np.ascontiguousarray(
                lamp[2 * c:2 * c + 2].reshape(1, HPC)),
        })
    return in_maps


def _run(inputs, trace=False):
    if "nc" not in _CACHE:
        _CACHE["nc"] = _build()
    nc = _CACHE["nc"]
    in_maps = _prep(**inputs)
    res = run_bass_kernel_spmd(nc, in_maps, core_ids=list(range(NCORES)),
                               trace=trace)
    out = np.concatenate([res.results[c]["o_out"] for c in range(NCORES)],
                         axis=0)
    return out.reshape(1, S, D), res


def kernel(**inputs):
    out, _ = _run(inputs)
    return out


# revision 41
# speedup vs baseline: 1.0226x; 1.0226x over previous
"""Differential attention (GQA + RoPE) Bass/Tile kernel for 8 TRN2 NeuronCores.

Sharding: tensor-parallel over the 16 query heads (2 per core, kv head c//2),
Wq/Wk/Wv column-sharded per core; attention output exchanged with an on-device
AllToAll into sequence shards; o_proj row-parallel per sequence shard with the
full Wo on every core; host concatenates the 8 row shards.

Layout notes:
 - x is passed transposed (xT [D, S]) so the contraction dim of every
   projection matmul lands on SBUF partitions.
 - Wq/Wk columns are permuted per head so RoPE's interleaved complex pairs
   become contiguous blocks [x0_A | x1_A | x0_B | x1_B] (A = freqs 0..31,
   B = freqs 32..63).  Attention scores are invariant to any per-half channel
   permutation applied consistently to q and k.
 - Softmax is computed max-free (scores for this problem are within ±6, far
   inside fp16/exp range); the row sum rides the AV matmul as an extra N=1
   matmul against a ones vector, reusing the loaded P^T stationary tile.
 - a1 - lam*a2 is folded linearly: out = (u1*inv_r1 - lam*u2*inv_r2) * 0.5.
"""

import numpy as np
from contextlib import ExitStack

import concourse.bacc as bacc
import concourse.tile as tile
from concourse import mybir
from concourse.bass_utils import run_bass_kernel_spmd

S = 2048
D = 2048
H = 16
KV = 4
HD = 128
HALF = 64
NCORES = 8
HPC = H // NCORES      # 2 query heads per core
P = 128
NT = S // P            # 16 tiles of 128 along s/t
NSC = 4                # s-chunks of 512
SCW = 512
DT = D // P            # 16 tiles along contraction dim
SROWS = S // NCORES    # 256 output rows per core
SCALE = 1.0 / 8.0      # 1/sqrt(HALF)
OUT_SCALE = 0.5        # 1 - lambda_init
NEG = -1.0e9

f32 = mybir.dt.float32
f32r = mybir.dt.float32r
f16 = mybir.dt.float16

_CACHE = {}


def _build():
    nc = bacc.Bacc("TRN2", target_bir_lowering=False, debug=False,
                   num_devices=NCORES)
    xT = nc.declare_dram_parameter("xT", [D, S], f16, isOutput=False)
    wall = nc.declare_dram_parameter("wall", [D, 4 * P], f16, isOutput=False)
    wo = nc.declare_dram_parameter("wo", [D, D], f16, isOutput=False)
    cosT = nc.declare_dram_parameter("cosT", [P, S], f16, isOutput=False)
    sinT = nc.declare_dram_parameter("sinT", [P, S], f16, isOutput=False)
    masks = nc.declare_dram_parameter("masks", [P, 4 * SCW], f16, isOutput=False)
    ident = nc.declare_dram_parameter("ident", [P, P], f32, isOutput=False)
    lam = nc.declare_dram_parameter("lam", [1, HPC], f32, isOutput=False)
    o_out = nc.declare_dram_parameter("o_out", [SROWS, D], f32, isOutput=True)

    rg = [list(range(NCORES))]

    with tile.TileContext(nc) as tc, ExitStack() as ctx:
        const = ctx.enter_context(tc.tile_pool(name="const", bufs=1))
        dram = ctx.enter_context(tc.tile_pool(name="dram", bufs=1, space="DRAM"))

        # rows 0..63 and 64..127 both hold freqs 0..63, so every rope
        # operand pair sees equal SBUF base partitions
        cos_sb = const.tile([P, S], f16)
        nc.gpsimd.dma_start(out=cos_sb[:, :], in_=cosT[:, :])
        sin_sb = const.tile([P, S], f16)
        nc.gpsimd.dma_start(out=sin_sb[:, :], in_=sinT[:, :])
        mask_sb = const.tile([P, 4 * SCW], f16)
        nc.gpsimd.dma_start(out=mask_sb[:, :], in_=masks[:, :])
        id_sb = const.tile([P, P], f32)
        nc.gpsimd.dma_start(out=id_sb[:, :], in_=ident[:, :])
        lam_sb = const.tile([1, HPC], f32)
        nc.gpsimd.dma_start(out=lam_sb[:, :], in_=lam[:, :])
        lam_sig = const.tile([1, HPC], f32)
        nc.scalar.activation(lam_sig[:, :], lam_sb[:, :],
                             mybir.ActivationFunctionType.Sigmoid)
        lam_b = const.tile([P, HPC], f32)
        nc.gpsimd.partition_broadcast(lam_b[:, :], lam_sig[:, :])

        # Persistent per-core tensors: projected qT/kT (rope applied) per head,
        # kT, vT (fp32, channel-major) and v16 (fp16, t-major for AV rhs).
        qkvp = ctx.enter_context(tc.tile_pool(name="qkvp", bufs=1))
        qkv = [qkvp.tile([P, S], f16, name=f"qkv{j}") for j in range(2)]
        # k with only half-A (resp. half-B) channels non-zero, so the score
        # matmuls contract over the full 128 partitions with no slicing
        kA = qkvp.tile([P, S], f16, name="kA")
        kB = qkvp.tile([P, S], f16, name="kB")
        nc.vector.memset(kA[:, :], 0.0)
        nc.vector.memset(kB[:, :], 0.0)
        vT32 = qkvp.tile([P, S], f32)
        # v in t-major fp16, one 136-wide group per t-tile:
        # cols [136jt, 136jt+128) = v, col 136jt+128 = 2.0 -- the rowsum
        # rider; pre-doubled sums make their reciprocal carry the 0.5 scale
        VG = 136
        v16 = qkvp.tile([P, NT * VG], f16)
        nc.vector.memset(v16[:, :].rearrange("p (jt g) -> p jt g", g=VG)
                         [:, :, 128:129], 2.0)

        # ---- Stage 1: fused qkv projection (+RoPE on eviction) ----
        with tc.tile_pool(name="wall_p", bufs=1) as wall_pool, \
             tc.tile_pool(name="xt_p", bufs=2) as xt_pool, \
             tc.tile_pool(name="rtmp", bufs=4) as rtmp, \
             tc.tile_pool(name="qscr", bufs=6) as qscr, \
             tc.tile_pool(name="ps1", bufs=4, space="PSUM") as ps1:
            # one DMA for the whole weight block: column group dt holds
            # wall[dt*128:(dt+1)*128, :] -> [128, 16*512]
            w_sb = wall_pool.tile([P, DT * 4 * P], f16, name="w_sb")
            for g in range(4):
                gdt = slice(g * 4, (g + 1) * 4)
                nc.scalar.dma_start(
                    out=w_sb[:, g * 4 * 4 * P:(g + 1) * 4 * 4 * P].rearrange(
                        "p (dt j) -> p dt j", dt=4),
                    in_=wall.ap().rearrange("(dt p) j -> p dt j", p=P)
                    [:, gdt, :])

            for sc in range(NSC):
                # one DMA per s-chunk: column group dt holds
                # xT[dt*128:(dt+1)*128, sc*512:(sc+1)*512]
                xts = xt_pool.tile([P, DT * SCW], f16, name="xt", tag="xt")
                for g in range(4):
                    gdt = slice(g * 4, (g + 1) * 4)
                    nc.sync.dma_start(
                        out=xts[:, g * 4 * SCW:(g + 1) * 4 * SCW].rearrange(
                            "p (dt f) -> p dt f", dt=4),
                        in_=xT[:, sc * SCW:(sc + 1) * SCW].rearrange(
                            "(dt p) f -> p dt f", p=P)[:, gdt, :])
                for j in (2, 0, 1, 3):
                    psum_p = ps1.tile([P, SCW], f32, name="psum_p", tag="p1")
                    for dt_ in range(DT):
                        nc.tensor.matmul(
                            psum_p[:, :],
                            w_sb[:, dt_ * 4 * P + j * P:
                                 dt_ * 4 * P + (j + 1) * P],
                            xts[:, dt_ * SCW:(dt_ + 1) * SCW],
                            start=(dt_ == 0), stop=(dt_ == DT - 1))
                    ssl = slice(sc * SCW, (sc + 1) * SCW)
                    # psum rows: [x0 (freqs 0..63) | x1 (freqs 0..63)]
                    tags = ("t0", "t1") if j < 2 else ("kt0", "kt1")
                    t0 = rtmp.tile([P, SCW], f16, name="t0", tag=tags[0])
                    t1 = rtmp.tile([P, SCW], f16, name="t1", tag=tags[1])
                    if j < 3:
                        # evict on the scalar engine so the PSUM bank frees
                        # fast; rope then runs SBUF->SBUF on DVE
                        xsc = qscr.tile([P, SCW], f16, name="xsc", tag="xsc")
                        nc.scalar.copy(xsc[:, :], psum_p[:, :])
                        psum_p = xsc
                    if j < 2:
                        # q rope at full 64-row width:
                        # rows 0..63 = r0, rows 64..127 = r1
                        q_t = qkv[j]
                        nc.vector.tensor_mul(t0[0:64, :], psum_p[64:128, :],
                                             sin_sb[64:128, ssl])
                        nc.vector.tensor_mul(q_t[0:64, ssl], psum_p[0:64, :],
                                             cos_sb[0:64, ssl])
                        nc.vector.tensor_sub(q_t[0:64, ssl], q_t[0:64, ssl],
                                             t0[0:64, :])
                        nc.vector.tensor_mul(t1[64:128, :], psum_p[0:64, :],
                                             sin_sb[0:64, ssl])
                        nc.vector.tensor_mul(q_t[64:128, ssl],
                                             psum_p[64:128, :],
                                             cos_sb[64:128, ssl])
                        nc.vector.tensor_add(q_t[64:128, ssl],
                                             q_t[64:128, ssl], t1[64:128, :])
                    elif j == 2:
                        # k rope scattered into kA (half-A rows) / kB (half-B)
                        for hf, kt in ((0, kA), (1, kB)):
                            fr = slice(32 * hf, 32 * hf + 32)
                            r1 = slice(64 + 32 * hf, 64 + 32 * hf + 32)
                            x0 = psum_p[fr, :]
                            x1 = psum_p[r1, :]
                            eng = nc.vector
                            eng.tensor_mul(t0[fr, :], x1, sin_sb[r1, ssl])
                            eng.tensor_mul(kt[fr, ssl], x0, cos_sb[fr, ssl])
                            eng.tensor_sub(kt[fr, ssl], kt[fr, ssl],
                                           t0[fr, :])
                            eng.tensor_mul(t1[r1, :], x0, sin_sb[fr, ssl])
                            eng.tensor_mul(kt[r1, ssl], x1, cos_sb[r1, ssl])
                            eng.tensor_add(kt[r1, ssl], kt[r1, ssl],
                                           t1[r1, :])
                    else:
                        nc.scalar.copy(vT32[:, ssl], psum_p[:, :])

        # ---- Stage 1.5: transpose v to t-major fp16 ----
        with tc.tile_pool(name="pst", bufs=1, space="PSUM") as pst:
            for jt in range(NT):
                ps_t = pst.tile([P, P], f32, name="ps_vt", tag="vt")
                nc.tensor.transpose(ps_t[:, :],
                                    vT32[:, jt * P:(jt + 1) * P], id_sb[:, :])
                nc.scalar.copy(v16[:, jt * 136:jt * 136 + P], ps_t[:, :])

        # prefetch full Wo (fp16) early on the scalar DMA queue so the
        # o_proj partials can start the moment the first AllToAll lands
        wo_pool = ctx.enter_context(tc.tile_pool(name="wo_p", bufs=1))
        wos_l = []
        for dc in range(4):
            wos = wo_pool.tile([P, H * SCW], f16, name="wos", tag=f"wos{dc}")
            nc.gpsimd.dma_start(
                out=wos[:, :].rearrange("p (ht f) -> p ht f", ht=H),
                in_=wo[:, dc * SCW:(dc + 1) * SCW].rearrange(
                    "(ht p) f -> p ht f", p=P))
            wos_l.append(wos)

        # ---- Stage 2: differential attention per (head, s-chunk) ----
        attnp = ctx.enter_context(tc.tile_pool(name="attnp", bufs=1))
        attnT = [attnp.tile([P, S], f16, name=f"attnT{h}") for h in range(HPC)]

        # per-head AllToAll bounce buffers (issued as soon as head h is done,
        # so the first exchange overlaps the second head's compute)
        sec = P * SROWS  # elems per (core, head) section
        bounce_in = [dram.tile([NCORES * sec], f16, name=f"bounce_in{h}")
                     for h in range(HPC)]
        bounce_out = [dram.tile([NCORES * sec], f16, name=f"bounce_out{h}")
                      for h in range(HPC)]

        misc_ps = ctx.enter_context(
            tc.tile_pool(name="misc_ps", bufs=2, space="PSUM"))
        with tc.tile_pool(name="expst_p", bufs=2) as expst_pool, \
             tc.tile_pool(name="st_p", bufs=2, space="PSUM") as st_pool, \
             tc.tile_pool(name="u_p", bufs=1, space="PSUM") as u_pool, \
             tc.tile_pool(name="cmb", bufs=4) as cmb:
            UG = 136
            for h in range(HPC):
                for sc in range(NSC):
                    njt = 4 * sc + 4
                    ps_u = [None, None]
                    for hf in range(2):
                        k_t = kA if hf == 0 else kB
                        expst = expst_pool.tile([P, njt * SCW], f16,
                                                name="expst", tag="expst",
                                                bufs=4)
                        for jt in range(njt):
                            ps_st = st_pool.tile([P, SCW], f32,
                                                 name="ps_st", tag="st")
                            nc.tensor.matmul(
                                ps_st[:, :],
                                k_t[:, jt * P:(jt + 1) * P],
                                qkv[h][:, sc * SCW:(sc + 1) * SCW],
                                start=True, stop=True)
                            # columns below 128*m are never read by the AV
                            # loop (fully-masked): restrict mask+exp to live.
                            # Causality is applied as a 0/1 fp16 multiply on
                            # the exp output (cheaper on DVE than the fp32
                            # PSUM -1e9 add; the rowsum rider sums the same
                            # zeroed tile, so the math is unchanged).
                            lo = 0
                            if jt >= 4 * sc:
                                m = jt % 4
                                lo = P * m
                            esl = expst[:, jt * SCW + lo:(jt + 1) * SCW]
                            nc.scalar.activation(
                                esl, ps_st[:, lo:],
                                mybir.ActivationFunctionType.Exp, scale=SCALE)
                            if jt >= 4 * sc and lo < SCW:
                                nc.vector.tensor_mul(
                                    esl, esl,
                                    mask_sb[:, m * SCW + lo:(m + 1) * SCW])
                        # two banks per half: [u(128) | r(1) | pad] x2
                        ps_u[hf] = [
                            u_pool.tile([P, 2 * UG], f32,
                                        name=f"ps_u{hf}{qq}", tag=f"u{hf}{qq}")
                            for qq in range(2)]
                        for q_ in range(4):
                            js = 4 * sc + q_
                            put = ps_u[hf][q_ // 2]
                            off = UG * (q_ % 2)
                            for jt in range(js + 1):
                                lhs = expst[:, jt * SCW + q_ * P:
                                            jt * SCW + q_ * P + P]
                                nc.tensor.matmul(
                                    put[:, off:off + 129],
                                    lhs, v16[:, jt * VG:jt * VG + 129],
                                    start=(jt == 0), stop=(jt == js))
                    # combine: attn = 0.5*(u1*inv_r1 - lam*u2*inv_r2)
                    inv = cmb.tile([P, 8], f32, name="inv", tag="inv")
                    for hf in range(2):
                        for q_ in range(4):
                            nc.vector.reciprocal(
                                inv[:, 4 * hf + q_:4 * hf + q_ + 1],
                                ps_u[hf][q_ // 2][:, UG * (q_ % 2) + 128:
                                                  UG * (q_ % 2) + 129])
                    for q_ in range(4):
                        js = 4 * sc + q_
                        u0 = ps_u[0][q_ // 2][:, UG * (q_ % 2):
                                              UG * (q_ % 2) + P]
                        u1 = ps_u[1][q_ // 2][:, UG * (q_ % 2):
                                              UG * (q_ % 2) + P]
                        sc2 = cmb.tile([P, 1], f32, name="sc2", tag="sc2")
                        nc.vector.tensor_scalar_mul(
                            sc2[:, :], inv[:, 4 + q_:5 + q_],
                            lam_b[:, h:h + 1])
                        tmp2 = cmb.tile([P, P], f32, name="tmp2", tag="tmp2")
                        nc.vector.tensor_scalar_mul(tmp2[:, :], u1, sc2[:, :])
                        attn_sl = cmb.tile([P, P], f32, name="attn_sl",
                                           tag="attn_sl")
                        nc.vector.scalar_tensor_tensor(
                            attn_sl[:, :], u0,
                            inv[:, q_:q_ + 1], tmp2[:, :],
                            mybir.AluOpType.mult, mybir.AluOpType.subtract)
                        ps_t = misc_ps.tile([P, P], f32, name="ps_at",
                                            tag="misc")
                        nc.tensor.transpose(ps_t[:, :], attn_sl[:, :],
                                            id_sb[:, :])
                        nc.vector.tensor_copy(attnT[h][:, js * P:(js + 1) * P],
                                              ps_t[:, :])
                # ---- Stage 3 (per head): AllToAll into sequence shards ----
                if sc == NSC - 1:
                    nc.gpsimd.dma_start(
                        out=bounce_in[h][:].rearrange(
                            "(d p f) -> p d f", d=NCORES, f=SROWS),
                        in_=attnT[h][:, :].rearrange(
                            "p (d f) -> p d f", f=SROWS))
                    nc.gpsimd.collective_compute(
                        "AllToAll", mybir.AluOpType.bypass, replica_groups=rg,
                        ins=[bounce_in[h][:]], outs=[bounce_out[h][:]])

        # ---- Stage 4: o_proj over the local 256 rows ----
        with tc.tile_pool(name="aT_p", bufs=1) as aT_pool, \
             tc.tile_pool(name="o_p", bufs=4) as o_pool, \
             tc.tile_pool(name="ps4", bufs=2, space="PSUM") as ps4:
            aTl = []
            for h in range(HPC):
                a_t = aT_pool.tile([P, NCORES * SROWS], f16, name=f"aT{h}")
                nc.gpsimd.dma_start(
                    out=a_t[:, :].rearrange("p (d f) -> p d f", d=NCORES),
                    in_=bounce_out[h][:].rearrange(
                        "(d p f) -> p d f", d=NCORES, f=SROWS))
                aTl.append(a_t)
            # head-0 sections only need the first AllToAll: run ALL of their
            # partial o_proj groups while the second exchange is in flight.
            o_es = {}
            for dc in range(4):
                for st_ in range(2):
                    ps_e = misc_ps.tile([P, SCW], f32, name="ps_e", tag="misc")
                    for i, ht in enumerate(range(0, H, 2)):
                        nc.tensor.matmul(
                            ps_e[:, :],
                            aTl[0][:, (ht // 2) * SROWS + st_ * P:
                                   (ht // 2) * SROWS + (st_ + 1) * P],
                            wos_l[dc][:, ht * SCW:(ht + 1) * SCW],
                            start=(i == 0), stop=(i == H // 2 - 1))
                    o_e = o_pool.tile([P, SCW], f32, name="o_e",
                                      tag=f"o_e{dc}{st_}", bufs=1)
                    nc.vector.tensor_copy(o_e[:, :], ps_e[:, :])
                    o_es[(dc, st_)] = o_e
            # keep the PE (HAM) warm across the second AllToAll's wait
            # window so the head-1 pass below starts at full clock; the
            # result is parked in DRAM and never read.
            warm_ps = misc_ps.tile([P, SCW], f32, name="warm", tag="misc")
            for w_ in range(40):
                nc.tensor.matmul(warm_ps[:, :], v16[:, 0:P],
                                 aTl[0][:, 0:SCW], start=True, stop=True)
            warm_sb = o_pool.tile([P, SCW], f32, name="warm_sb",
                                  tag="warm_sb", bufs=1)
            nc.vector.tensor_copy(warm_sb[:, :], warm_ps[:, :])
            warm_dram = dram.tile([P * SCW], f32, name="warm_dram")
            nc.gpsimd.dma_start(
                out=warm_dram[:].rearrange("(p f) -> p f", f=SCW),
                in_=warm_sb[:, :])
            for dc in range(4):
                for st_ in range(2):
                    ps_o = ps4.tile([P, SCW], f32, name="ps_o", tag="o")
                    for i, ht in enumerate(range(1, H, 2)):
                        nc.tensor.matmul(
                            ps_o[:, :],
                            aTl[1][:, (ht // 2) * SROWS + st_ * P:
                                   (ht // 2) * SROWS + (st_ + 1) * P],
                            wos_l[dc][:, ht * SCW:(ht + 1) * SCW],
                            start=(i == 0), stop=(i == H // 2 - 1))
                    o_sb = o_pool.tile([P, SCW], f32, name="o_sb", tag="o_sb")
                    nc.vector.tensor_add(o_sb[:, :], ps_o[:, :],
                                         o_es[(dc, st_)][:, :])
                    nc.sync.dma_start(
                        out=o_out[st_ * P:(st_ + 1) * P,
                                  dc * SCW:(dc + 1) * SCW],
                        in_=o_sb[:, :])

    nc.compile()
    return nc


def _prep(x, freqs_cos, freqs_sin, Wq, Wk, Wv, Wo, lambda_param):
    """Host-side sharding/layout prep. Returns per-core input maps."""
    x2 = np.asarray(x, np.float32).reshape(S, D)
    xT = np.ascontiguousarray(x2.T.astype(np.float16))
    cosT = np.asarray(freqs_cos, np.float32).T
    sinT = np.asarray(freqs_sin, np.float32).T
    cosT = np.ascontiguousarray(
        np.concatenate([cosT, cosT], axis=0).astype(np.float16))
    sinT = np.ascontiguousarray(
        np.concatenate([sinT, sinT], axis=0).astype(np.float16))
    Wq = np.asarray(Wq, np.float32)
    Wk = np.asarray(Wk, np.float32)
    Wv = np.asarray(Wv, np.float32)
    Wo16 = np.ascontiguousarray(np.asarray(Wo, np.float32).astype(np.float16))
    lamp = np.asarray(lambda_param, np.float32)

    # de-interleave complex pairs: [x0 (freqs 0..63) | x1 (freqs 0..63)]
    perm = np.concatenate([
        2 * np.arange(64), 2 * np.arange(64) + 1]).astype(np.int64)

    # causal mask variants for the 4 in-chunk diagonal positions
    t_rel = np.arange(P)[:, None]
    s_rel = np.arange(SCW)[None, :]
    mask_all = np.empty((P, 4 * SCW), np.float16)
    for m in range(4):
        mask_all[:, m * SCW:(m + 1) * SCW] = np.where(
            P * m + t_rel <= s_rel, 1.0, 0.0)

    ident = np.eye(P, dtype=np.float32)

    in_maps = []
    for c in range(NCORES):
        g = c // 2
        cols = []
        for h in (2 * c, 2 * c + 1):
            cols.append(Wq[:, h * HD:(h + 1) * HD][:, perm])
        cols.append(Wk[:, g * HD:(g + 1) * HD][:, perm])
        cols.append(Wv[:, g * HD:(g + 1) * HD])
        wall = np.ascontiguousarray(
            np.concatenate(cols, axis=1).astype(np.float16))
        in_maps.append({
            "xT": xT,
            "wall": wall,
            "wo": Wo16,
            "cosT": cosT,
            "sinT": sinT,
            "masks": mask_all,
            "ident": ident,
            "lam": np.ascontiguousarray(
                lamp[2 * c:2 * c + 2].reshape(1, HPC)),
        })
    return in_maps


def _run(inputs, trace=False):
    if "nc" not in _CACHE:
        _CACHE["nc"] = _build()
    nc = _CACHE["nc"]
    in_maps = _prep(**inputs)
    res = run_bass_kernel_spmd(nc, in_maps, core_ids=list(range(NCORES)),
                               trace=trace)
    out = np.concatenate([res.results[c]["o_out"] for c in range(NCORES)],
                         axis=0)
    return out.reshape(1, S, D), res


def kernel(**inputs):
    out, _ = _run(inputs)
    return out
